# revision 1
# baseline (speedup 1.0000x reference)
"""Trainium2 Bass kernel for ColorImageLoss (gaussian-blur + bilinear grid
sample + MSE), data-parallel over batch across 8 NeuronCores.

Key idea: the loss only reads the blurred image at 64 sample points per
image.  Each bilinear sample needs a 2x2 patch of blurred pixels; the 7-tap
separable blur support of those pixels is an 8x8 patch of the *original*
image.  Reflect padding folds into per-sample 8-tap row/col weight vectors
(reflected tap indices provably stay inside the clamped 8-wide window
[clamp(x0-3,0,W-8), +8)).  So per sample we indirect-DMA-gather an 8x8x3
patch and compute  target_c = vw^T @ Patch_c @ hw  on device, then the MSE
partial sum.  HBM traffic: ~200KB/core instead of 12.6MB/core.
"""

import os
import sys

import numpy as np

for _p in ("/opt/trn_rl_repo", "/root/.axon_site/_ro/trn_rl_repo"):
    if os.path.isdir(_p) and _p not in sys.path:
        sys.path.insert(0, _p)

import concourse.bass as bass
import concourse.mybir as mybir
import concourse.tile as tile
from concourse.bass_utils import run_bass_kernel_spmd

# Problem geometry (hardcoded per contract)
B, L, NCH, H, W = 32, 64, 3, 512, 512
NCORES = 8
BPC = B // NCORES            # images per core
NS = BPC * L                 # samples per core (256)
P = 128                      # SBUF partitions
SLOTS = NS // P              # 2 sample slots per partition
KS = 7                       # blur taps
IMG_ELEMS = BPC * NCH * H * W

f32 = mybir.dt.float32
i32 = mybir.dt.int32
Alu = mybir.AluOpType
Ax = mybir.AxisListType

# meta tensor per-partition layout (f32 columns)
O_POS = 0            # [SLOTS, 2] (x, y)                -> 4
O_COL = 4            # [SLOTS, 3] color                 -> 6
O_JM3 = 10           # [4, 7] j-3 replicated per var    -> 28
O_KK = 38            # [7] blur kernel                  -> 7
O_IB = 45            # [SLOTS, 3, 8] gather index base  -> 48
O_IOTA8 = 93         # [8] 0..7                         -> 8
META_W = 101


def _gauss_kernel_np():
    x = (np.arange(KS, dtype=np.float32) - (KS - 1) / 2).astype(np.float32)
    k = np.exp(-0.5 * (x / np.float32(1.0)) ** 2).astype(np.float32)
    return (k / k.sum()).astype(np.float32)


def _fap(t, dims, extra_offset=0):
    """AP over tile `t` keeping its partition dim, replacing free dims.

    dims: list of [step, count] in elements; step 0 broadcasts.
    """
    base = t[:] if hasattr(t, "tile") else t
    return bass.AP(
        base.tensor, base.offset + extra_offset,
        [list(base.ap[0])] + [list(d) for d in dims],
    )


def split_multi_waits(nc):
    """This walrus encodes at most ONE sync wait per TPB instruction.  Hoist
    extra waits onto same-engine NoOps inserted directly before the
    instruction (the sequencer executes waits in queue order, so semantics
    are identical)."""
    n_split = 0
    for f in nc.m.functions:
        for blk in f.blocks:
            insts = blk.instructions
            i = 0
            while i < len(insts):
                inst = insts[i]
                si = inst.sync_info
                if si is not None and si.on_wait is not None and len(si.on_wait) > 1:
                    waits = list(si.on_wait)
                    for w in waits[:-1]:
                        nop = mybir.InstNoOp(
                            name=f"{inst.name}-wsplit{n_split}",
                            engine=inst.engine,
                            ins=[],
                            outs=[],
                            sync_info=mybir.SyncInfo(on_wait=[w], on_update=[]),
                        )
                        nc.register_instruction(nop, overwrite=True)
                        insts.insert(i, nop)
                        i += 1
                        n_split += 1
                    inst.sync_info = mybir.SyncInfo(
                        on_wait=[waits[-1]], on_update=list(si.on_update or []))
                i += 1
    return n_split


def build_bass(debug_taps=False, repeat=1, skip_gather=False, skip_compute=False):
    nc = bass.Bass("TRN2")

    img = nc.dram_tensor("img", [IMG_ELEMS, 1], f32, kind="ExternalInput")
    meta = nc.dram_tensor("meta", [P, META_W], f32, kind="ExternalInput")
    partial = nc.dram_tensor("partial", [P, 1], f32, kind="ExternalOutput")

    taps = []

    def tap(name, t, width):
        if not debug_taps:
            return
        d = nc.dram_tensor(f"tap_{name}", [P, width], f32, kind="ExternalOutput")
        taps.append((name, t, d, width))

    with tile.TileContext(nc) as tc:
        for _rep in range(repeat):
            with tc.tile_pool(name=f"main{_rep}", bufs=1) as pool:
                m = pool.tile([P, META_W], f32)
                nc.sync.dma_start(out=m[:], in_=meta[:])

                # ---- positions -> x,y (reference op order, f32) ----
                # gx = 2p-1 ; v = ((gx+1)*W - 1)*0.5 ; clip [0, W-1]
                xy = pool.tile([P, SLOTS, 2], f32)
                pos_ap = _fap(m, [[2, SLOTS], [1, 2]], O_POS)
                nc.vector.tensor_scalar(xy[:], pos_ap, 2.0, -1.0, Alu.mult, Alu.add)
                nc.vector.tensor_scalar(xy[:], xy[:], 1.0, float(W), Alu.add, Alu.mult)
                nc.vector.tensor_scalar(xy[:], xy[:], -1.0, 0.5, Alu.add, Alu.mult)
                nc.vector.tensor_scalar(xy[:], xy[:], 0.0, float(W - 1), Alu.max, Alu.min)
                tap('xy', xy, 4)

                # ---- floor/frac ----
                # floor via the exact round-to-nearest trick: (x + 2^23) - 2^23
                # rounds to integer (f32 grid at 2^23 is 1.0); subtract the
                # rounded-up-past-x case.  Two separate instructions so each
                # result rounds to f32 in SBUF.
                rnd = pool.tile([P, SLOTS, 2], f32)
                nc.vector.tensor_scalar(rnd[:], xy[:], 8388608.0, None, Alu.add)
                nc.vector.tensor_scalar(rnd[:], rnd[:], -8388608.0, None, Alu.add)
                gtx = pool.tile([P, SLOTS, 2], f32)
                nc.vector.tensor_tensor(gtx[:], rnd[:], xy[:], op=Alu.is_gt)
                wxy = pool.tile([P, SLOTS, 2], f32)   # frac (wx, wy)
                fxy = pool.tile([P, SLOTS, 2], f32)   # floor (x0, y0)
                nc.vector.tensor_sub(fxy[:], rnd[:], gtx[:])
                nc.vector.tensor_sub(wxy[:], xy[:], fxy[:])
                tap('fxy', fxy, 4)
                tap('wxy', wxy, 4)

                # ---- p4 [P, SLOTS, 2axis, 2which]: (x0, x1, y0, y1) ----
                p4 = pool.tile([P, SLOTS, 2, 2], f32)
                nc.vector.tensor_copy(_fap(p4, [[4, SLOTS], [2, 2], [1, 1]]), fxy[:])
                nc.vector.tensor_scalar(
                    _fap(p4, [[4, SLOTS], [2, 2], [1, 1]], 1),
                    fxy[:], 1.0, float(W - 1), Alu.add, Alu.min)
                tap('p4', p4, 8)

                # ---- window starts s = clamp(floor-3, 0, W-8) [P, SLOTS, 2] ----
                s_t = pool.tile([P, SLOTS, 2], f32)
                nc.vector.tensor_scalar(s_t[:], fxy[:], -3.0, 0.0, Alu.add, Alu.max)
                nc.vector.tensor_scalar(s_t[:], s_t[:], float(W - 8), None, Alu.min)
                tap('s_t', s_t, 4)

                # ---- gather indices [P, SLOTS, 3ch, 8row] ----
                rc = pool.tile([P, SLOTS], f32)       # sy*W + sx
                nc.vector.tensor_scalar(
                    rc[:], _fap(s_t, [[2, SLOTS], [1, 1]], 1), float(W), None, Alu.mult)
                nc.vector.tensor_tensor(
                    rc[:], rc[:], _fap(s_t, [[2, SLOTS], [1, 1]]), op=Alu.add)
                idxf = pool.tile([P, SLOTS, NCH, 8], f32)
                ib = _fap(m, [[24, SLOTS], [8, NCH], [1, 8]], O_IB)
                rc_b = _fap(rc, [[1, SLOTS], [0, NCH * 8]])
                nc.vector.tensor_tensor(
                    _fap(idxf, [[NCH * 8, SLOTS], [1, NCH * 8]]), ib, rc_b, op=Alu.add)
                tap('idxf', idxf, 48)
                idx = pool.tile([P, SLOTS, NCH, 8], i32)
                nc.vector.tensor_copy(idx[:], idxf[:])

                # ---- indirect gather: 8 contiguous pixels per index ----
                # HW SWDGE pairs ONE index per partition-row descriptor per call
                # (probe-verified; multi-index-per-partition layouts misbehave).
                # So issue one call per segment column: each call gathers one
                # 8-px run per partition using a [P, 1] index slice.
                patches = pool.tile([P, SLOTS, NCH, 8, 8], f32)
                for seg in range(0 if skip_gather else SLOTS * NCH * 8):
                    nc.gpsimd.indirect_dma_start(
                        out=_fap(patches, [[1, 8]], 8 * seg),
                        out_offset=None,
                        in_=img[:],
                        in_offset=bass.IndirectOffsetOnAxis(
                            ap=_fap(idx, [[1, 1]], seg), axis=0),
                    )


                # ---- raw tap positions T [P, SLOTS, 4var, 7] = p4 + (j-3) ----
                t_t = pool.tile([P, SLOTS, 4, KS], f32)
                p4_b = _fap(p4, [[4, SLOTS], [1, 4], [0, KS]])
                jm3 = _fap(m, [[0, SLOTS], [KS, 4], [1, KS]], O_JM3)
                nc.vector.tensor_add(t_t[:], p4_b, jm3)
                tap('t_t', t_t, 56)

                # ---- reflect: R = min(abs(T), 2*(W-1) - T); abs = max(T, -T) ----
                neg = pool.tile([P, SLOTS, 4, KS], f32)
                a_t = pool.tile([P, SLOTS, 4, KS], f32)
                b_t = pool.tile([P, SLOTS, 4, KS], f32)
                nc.vector.tensor_scalar(neg[:], t_t[:], -1.0, None, Alu.mult)
                nc.vector.tensor_tensor(a_t[:], t_t[:], neg[:], op=Alu.max)
                nc.vector.tensor_scalar(
                    b_t[:], t_t[:], -1.0, float(2 * (W - 1)), Alu.mult, Alu.add)
                r_t = pool.tile([P, SLOTS, 4, KS], f32)
                nc.vector.tensor_tensor(r_t[:], a_t[:], b_t[:], op=Alu.min)
                tap('r_t', r_t, 56)

                # ---- window-relative tap Z = R - s(axis)  in [0,8) ----
                z_t = pool.tile([P, SLOTS, 4, KS], f32)
                s_b = _fap(s_t, [[2, SLOTS], [1, 2], [0, 2 * KS]])
                r_v = _fap(r_t, [[4 * KS, SLOTS], [2 * KS, 2], [1, 2 * KS]])
                z_v = _fap(z_t, [[4 * KS, SLOTS], [2 * KS, 2], [1, 2 * KS]])
                nc.vector.tensor_tensor(z_v, r_v, s_b, op=Alu.subtract)
                tap('z_t', z_t, 56)

                # ---- per-window-offset kernel weights K [P, SLOTS, 4var, 8] ----
                # K[v, u] = sum_j kk[j] * (Z[v, j] == u)
                eq = pool.tile([P, SLOTS * 4, 8, KS], f32)
                z_b = _fap(z_t, [[KS, SLOTS * 4], [0, 8], [1, KS]])
                iota_b = _fap(m, [[0, SLOTS * 4], [1, 8], [0, KS]], O_IOTA8)
                nc.vector.tensor_tensor(eq[:], z_b, iota_b, op=Alu.is_equal)
                kk_b = _fap(m, [[0, SLOTS * 4], [0, 8], [1, KS]], O_KK)
                nc.vector.tensor_tensor(eq[:], eq[:], kk_b, op=Alu.mult)
                tap('eqk', eq, 448)
                kw = pool.tile([P, SLOTS, 4, 8], f32)
                nc.vector.tensor_reduce(
                    out=_fap(kw, [[1, SLOTS * 4 * 8]]),
                    in_=eq[:], axis=Ax.X, op=Alu.add)
                tap('kw', kw, 64)

                # ---- bilinear weights ww [P, SLOTS, 2axis, 2which] ----
                ww = pool.tile([P, SLOTS, 2, 2], f32)
                nc.vector.tensor_copy(_fap(ww, [[4, SLOTS], [2, 2], [1, 1]], 1), wxy[:])
                nc.vector.tensor_scalar(
                    _fap(ww, [[4, SLOTS], [2, 2], [1, 1]]),
                    wxy[:], -1.0, 1.0, Alu.mult, Alu.add)

                # ---- vh = K * ww  -> summed over which -> axis taps [P,SLOTS,2,8]
                vh = pool.tile([P, SLOTS, 4, 8], f32)
                ww_b = _fap(ww, [[4, SLOTS], [1, 4], [0, 8]])
                nc.vector.tensor_tensor(vh[:], kw[:], ww_b, op=Alu.mult)
                hwv = pool.tile([P, SLOTS, 2, 8], f32)   # axis 0 = x taps, 1 = y taps
                vh0 = _fap(vh, [[32, SLOTS], [16, 2], [1, 8]])
                vh1 = _fap(vh, [[32, SLOTS], [16, 2], [1, 8]], 8)
                nc.vector.tensor_tensor(hwv[:], vh0, vh1, op=Alu.add)
                tap('hwv', hwv, 32)

                # ---- outer product wp[u,t] = vw[u]*hw[t] [P, SLOTS, 8, 8] ----
                wp = pool.tile([P, SLOTS, 8, 8], f32)
                vw_b = _fap(hwv, [[16, SLOTS], [1, 8], [0, 8]], 8)   # y taps (rows)
                hw_b = _fap(hwv, [[16, SLOTS], [0, 8], [1, 8]])      # x taps (cols)
                nc.vector.tensor_tensor(wp[:], vw_b, hw_b, op=Alu.mult)
                tap('wp', wp, 128)

                # ---- apply weights, reduce to target, MSE partial ----
                # Wait-splitter: compute instructions encode at most one sync
                # wait.  This copy's only dependency is the gather DMA, so it
                # absorbs the DMASW wait; the multiply below then only needs the
                # same-engine DVE chain wait.
                dummy = pool.tile([P, 1], f32)
                nc.vector.tensor_copy(dummy[:], _fap(patches, [[1, 1]]))
                tap('patches', patches, 384)
                tmp = pool.tile([P, SLOTS, NCH, 64], f32)
                wp_b = _fap(wp, [[64, SLOTS], [0, NCH], [1, 64]])
                pat_v = _fap(patches, [[NCH * 64, SLOTS], [64, NCH], [1, 64]])
                nc.vector.tensor_tensor(tmp[:], pat_v, wp_b, op=Alu.mult)
                tap('tmp', tmp, 384)
                tgt = pool.tile([P, SLOTS, NCH], f32)
                nc.vector.tensor_reduce(
                    out=_fap(tgt, [[1, SLOTS * NCH]]),
                    in_=_fap(tmp, [[64, SLOTS * NCH], [1, 64]]),
                    axis=Ax.X, op=Alu.add)
                tap('tgt', tgt, 6)
                diff = pool.tile([P, SLOTS, NCH], f32)
                col_ap = _fap(m, [[NCH, SLOTS], [1, NCH]], O_COL)
                nc.vector.tensor_tensor(diff[:], tgt[:], col_ap, op=Alu.subtract)
                sq = pool.tile([P, SLOTS, NCH], f32)
                nc.vector.tensor_tensor(sq[:], diff[:], diff[:], op=Alu.mult)
                part = pool.tile([P, 1], f32)
                nc.vector.tensor_reduce(
                    out=part[:], in_=_fap(sq, [[1, SLOTS * NCH]]), axis=Ax.X, op=Alu.add)

                nc.sync.dma_start(out=partial[:], in_=part[:])

                for _name, _t, _d, _w in taps:
                    nc.sync.dma_start(out=_d[:], in_=_fap(_t, [[1, _w]]))

    split_multi_waits(nc)
    return nc


def make_meta(pred_shard):
    """Build the per-core [P, META_W] meta tensor from the [BPC, L, 8]
    predictions shard.  Sample i = slot*P + p."""
    flat = np.ascontiguousarray(pred_shard.reshape(NS, 8).astype(np.float32))
    meta = np.zeros((P, META_W), dtype=np.float32)
    pos = flat[:, :2].reshape(SLOTS, P, 2).transpose(1, 0, 2)     # [P,SLOTS,2]
    col = flat[:, 5:8].reshape(SLOTS, P, 3).transpose(1, 0, 2)    # [P,SLOTS,3]
    meta[:, O_POS:O_POS + 4] = pos.reshape(P, 4)
    meta[:, O_COL:O_COL + 6] = col.reshape(P, 6)
    jm3 = np.tile((np.arange(KS, dtype=np.float32) - 3.0), 4)     # [4*7]
    meta[:, O_JM3:O_JM3 + 28] = jm3[None, :]
    meta[:, O_KK:O_KK + KS] = _gauss_kernel_np()[None, :]
    # gather index base: img(slot,p) * CH*H*W + c*H*W + u*W
    p_idx = np.arange(P)
    base = np.zeros((P, SLOTS, NCH, 8), dtype=np.float32)
    for slot in range(SLOTS):
        img_i = (slot * P + p_idx) // L                           # [P]
        for c in range(NCH):
            for u in range(8):
                base[:, slot, c, u] = (
                    img_i * (NCH * H * W) + c * (H * W) + u * W)
    meta[:, O_IB:O_IB + 48] = base.reshape(P, 48)
    meta[:, O_IOTA8:O_IOTA8 + 8] = np.arange(8, dtype=np.float32)[None, :]
    return meta


def make_in_maps(predictions, ref_imgs):
    in_maps = []
    for k in range(NCORES):
        img_shard = np.ascontiguousarray(
            ref_imgs[k * BPC:(k + 1) * BPC].astype(np.float32)).reshape(-1, 1)
        meta = make_meta(predictions[k * BPC:(k + 1) * BPC])
        in_maps.append({"img": img_shard, "meta": meta})
    return in_maps


_NC_CACHE = {}


def get_nc():
    if "nc" not in _NC_CACHE:
        _NC_CACHE["nc"] = build_bass()
    return _NC_CACHE["nc"]


def _reduce_results(res):
    total = np.float64(0.0)
    for r in res.results:
        total += np.float64(r["partial"].sum(dtype=np.float64))
    return np.float32(total / (B * L * NCH))


def kernel(predictions, ref_imgs):
    predictions = np.asarray(predictions)
    ref_imgs = np.asarray(ref_imgs)
    nc = get_nc()
    in_maps = make_in_maps(predictions, ref_imgs)
    res = run_bass_kernel_spmd(nc, in_maps, list(range(NCORES)))
    return _reduce_results(res)


def run_profiled(predictions, ref_imgs):
    """Like kernel(), but traces with neuron-profile; returns (loss, results)."""
    predictions = np.asarray(predictions)
    ref_imgs = np.asarray(ref_imgs)
    nc = get_nc()
    in_maps = make_in_maps(predictions, ref_imgs)
    res = run_bass_kernel_spmd(
        nc, in_maps, list(range(NCORES)), trace=True)
    return _reduce_results(res), res

